# revision 85
# baseline (speedup 1.0000x reference)
"""Trainium2 Bass kernel for CustomMHA (b=4, s=2048, d_model=1024, 16 heads).

Sharding: tensor-parallel over heads — each of the 8 cores computes QKV +
attention for its 2 heads, projects its 128 attention-output dims through its
rows of W_o into a full-width partial, and a per-batch ReduceScatter(add)
hands every core its 128-column slice of the final output.

Device-side structure (v3):
  - All matmul tiles are bf16 (1 cycle/row). Scores are computed transposed
    ([key, query] layout) so softmax needs no max pass; exp(x/8) is fused
    into the ACT scale. The attention kt-loop is ACT(exp)-bound at ~1038
    ns/kt; everything else injects into its PE slack as fine-grained work
    units drained from priority FIFOs (hot = av/tr, cold = QKV/V-transpose,
    proj_q = projection/reduce-scatter) under a credit model, with
    readiness gates enforcing emit-before-read for the engine FIFOs.
  - attn@V is output-transposed: stationary = exp-scores tile [keys, q128],
    moving = V^T tile [keys, dh+1] with a packed ones column; the
    denominator lands per-partition, so normalization is reciprocal+scale.
  - Head: the PE p-state ramp (needs ~3 us continuous work to hit 2.4 GHz)
    is burned with dummy identity transposes while the first DMAs land;
    b0 runs only Q(qc0)+K(tcl0) up front, K(tcl1..3) trickle in ahead of
    the kts that read them, and V/transposes/Q(qc1) defer into the loop,
    so the first exp starts ~15 us in (DMA-transfer-bound).
  - DMA: HWDGE costs a fixed ~625 ns per DMA instruction and transfers
    serialize, so x chunks load as one 8-d-chunk DMA and wqkv as one DMA.
  - Tail: the last iter's av chains are emitted so they prefetch during
    the final exps; PSUM evacuations alternate DVE/ACT (ACT idles once the
    exps are done); the final projections pair-write [128,1024] part DMAs;
    per-qc ReduceScatter chunks shorten the final collective.
"""

import numpy as np

import concourse.bass as bass
import concourse.tile as tile
from concourse import bacc, mybir
from concourse.masks import make_identity

F32 = mybir.dt.float32
F32R = mybir.dt.float32r
BF16 = mybir.dt.bfloat16
EXP = mybir.ActivationFunctionType.Exp

N_CORES = 8
D_MODEL = 1024
N_HEADS = 16
DH = 64
HPC = N_HEADS // N_CORES  # heads per core = 2

PHASE_MARKS = []
UNIT_LOG = []  # (label, id_watermark): instruction I-n belongs to the last
               # label whose watermark <= n. Populated during build_nc.


def build_nc(B=4, SEQ=2048, skip_collectives=False):
    """Build the SPMD Bass module (same program for all 8 cores)."""
    PHASE_MARKS.clear()
    UNIT_LOG.clear()
    TOK = B * SEQ
    QC = 1024                    # query chunk (scores/exp tile width)
    n_qc = SEQ // QC             # 2
    MS = 512                     # scores moving chunk (one PSUM bank)
    n_kt = SEQ // 128            # key tiles per (b, h)
    n_dc = D_MODEL // 128        # 8
    n_et = D_MODEL // 128        # 8
    n_tcb = SEQ // 512           # token chunks per batch
    n_qt = QC // 128             # qtiles per query chunk
    W2 = HPC * (DH + 1)          # 130 VT2 columns per token tile
    n_vt = TOK // 128

    nc = bacc.Bacc("TRN2", target_bir_lowering=False, debug=False,
                   num_devices=N_CORES)

    # xT is packed host-side as [128, d_chunk * TOK]: row r, col d*TOK+t
    # holds x[t, d*128+r], so one DMA covers all 8 d-chunks of a token chunk
    xT = nc.dram_tensor("xT", [128, n_dc * TOK], BF16,
                        kind="ExternalInput").ap()
    wqkv = nc.dram_tensor("wqkv", [128, n_dc * 3 * HPC * DH], BF16,
                          kind="ExternalInput").ap()
    wo = nc.dram_tensor("wo", [128, D_MODEL], BF16, kind="ExternalInput").ap()
    outT = nc.dram_tensor("outT", [128, TOK], BF16,
                          kind="ExternalOutput").ap()

    groups = [list(range(N_CORES))]

    with tile.TileContext(nc) as tc:
        import contextlib
        with contextlib.ExitStack() as ctx:
            res = ctx.enter_context(tc.tile_pool(name="resident", bufs=1))
            dram = ctx.enter_context(tc.tile_pool(name="dram", bufs=1,
                                                  space="DRAM"))
            xtp = ctx.enter_context(tc.tile_pool(name="xt", bufs=5))
            vbp = ctx.enter_context(tc.tile_pool(name="vb", bufs=3))
            atp = ctx.enter_context(tc.tile_pool(name="at", bufs=32))
            a2p = ctx.enter_context(tc.tile_pool(name="a2", bufs=3))
            # 9 bufs: the tail holds all 8 et-pairs of o_sb live at once
            osp = ctx.enter_context(tc.tile_pool(name="os", bufs=9))
            smallp = ctx.enter_context(tc.tile_pool(name="small", bufs=4))
            # PSUM budget (8 banks x 2KB): ps 2x4KB=4, shared po2/psT
            # 2x2KB=2, QKV pm 1, proj pp 1.
            psp = ctx.enter_context(tc.tile_pool(name="ps", bufs=2,
                                                 space="PSUM"))
            po2p = ctx.enter_context(tc.tile_pool(name="po2", bufs=2,
                                                  space="PSUM"))
            pmp = ctx.enter_context(tc.tile_pool(name="pm", bufs=1,
                                                 space="PSUM"))
            ppp = ctx.enter_context(tc.tile_pool(name="pp", bufs=1,
                                                 space="PSUM"))

            xt_tiles = {}
            v_tiles = {}
            qkv_psum = {}
            at_tiles = {}
            a2_tiles = {}

            # ---- resident tensors ----
            # x chunk DMAs for the first two token chunks go out ahead of the
            # weight loads so the first QKV matmuls aren't gated on the full
            # weight DMA
            xTv = xT.rearrange("p (d t) -> p d t", d=n_dc)
            xt00 = xtp.tile([128, n_dc * 512], BF16, tag="xt", name="xt00")
            xt00v = xt00[:].rearrange("p (d t) -> p d t", d=n_dc)
            # wqkv is packed host-side as [128, d_chunk * 384]; its chunk
            # DMAs interleave with the first x chunk halves so the first
            # QKV matmuls start after ~3 small transfers, keeping the PE
            # from a long cold idle (p-state ramp)
            W1 = 3 * HPC * DH
            w_all = res.tile([128, n_dc * W1], BF16, tag="wall")
            w_sb = [w_all[:, d * W1:(d + 1) * W1] for d in range(n_dc)]
            # HWDGE charges a fixed ~625 ns per DMA instruction and the DMA
            # engines serialize transfers, so batch: x00's first half, all
            # of wqkv in one DMA (Q(tcl0) q0/q1 need both), then the rest
            nc.sync.dma_start(xt00v[:, 0:4, :], xTv[:, 0:4, 0:512])
            nc.sync.dma_start(w_all[:], wqkv[:])
            nc.sync.dma_start(xt00v[:, 4:8, :], xTv[:, 4:8, 0:512])
            xt_tiles[(0, 0)] = xt00
            wo_sb = res.tile([128, D_MODEL], BF16, tag="wo")
            Q_sb = res.tile([128, TOK], BF16, tag="Q")
            K_sb = res.tile([128, TOK], BF16, tag="K")
            A_sb = res.tile([128, TOK], BF16, tag="A")
            VT2 = res.tile([128, n_vt * W2], BF16, tag="VT2")
            ident = res.tile([128, 128], BF16, tag="ident")
            onesrow = res.tile([128, 1], BF16, tag="ones1")
            make_identity(nc, ident[:])
            nc.gpsimd.memset(onesrow[:], 1.0)
            # write every ones-column of VT2 (col 64 of each 65-wide group)
            vt2_groups = VT2[:].rearrange("p (t c) -> p t c", c=DH + 1)
            nc.vector.tensor_copy(
                vt2_groups[:, :, DH:DH + 1],
                onesrow[:, None, :].broadcast_to([128, n_vt * HPC, 1]))
            # PE p-state warmup: the clock needs ~3 us of continuous PE
            # execution to reach 2.4 GHz; idle-waiting for the first x/w
            # DMAs would otherwise leave the whole b0 prefix at 1.2 GHz.
            # Dummy identity self-transposes keep the pipe busy meanwhile.
            warm = ppp.tile([128, 128], BF16, tag="pp", name="warm")
            for _ in range(80):
                nc.tensor.transpose(warm[:], ident[:], ident[:])

            part_d = {(b, qc): dram.tile([D_MODEL, QC], BF16,
                                         tag=f"part{b}.{qc}",
                                         name=f"part{b}.{qc}")
                      for b in range(B) for qc in range(n_qc)}
            rs_d = {(b, qc): dram.tile([128, QC], BF16, tag=f"rs{b}.{qc}",
                                       name=f"rs{b}.{qc}")
                    for b in range(B) for qc in range(n_qc)}

            # ---------------- work units ----------------
            def make_x_unit(bb, tcl):
                """DMA the 8 xT d-chunks of token chunk (bb, tcl) into SBUF."""
                def emit():
                    UNIT_LOG.append((f"x{bb}.{tcl}", nc.next_id()))
                    tci = bb * n_tcb + tcl
                    t = xtp.tile([128, n_dc * 512], BF16, tag="xt",
                                 name="xt")
                    tv = t[:].rearrange("p (d t) -> p d t", d=n_dc)
                    # one DMA for all 8 d-chunks: HWDGE charges a fixed
                    # ~625 ns per DMA instruction, and steady-state units
                    # are prefetched an iteration ahead anyway
                    nc.sync.dma_start(tv[:], xTv[:, :, tci * 512:(tci + 1) * 512])
                    xt_tiles[(bb, tcl)] = t
                return emit

            vt_emitted = {}  # bb -> V-transpose units emitted so far

            def make_t_unit(bb, tcl, j4):
                """Transpose one [128,128] V tile into VT2 (+ones layout)."""
                def emit():
                    UNIT_LOG.append((f"t{bb}.{tcl}.{j4}", nc.next_id()))
                    vt_emitted[bb] = vt_emitted.get(bb, 0) + 1
                    t_i = (bb * n_tcb + tcl) * 4 + j4
                    vt = v_tiles[(bb, tcl)]
                    psT = po2p.tile([128, 128], BF16, tag="po2", name="psT")
                    nc.tensor.transpose(
                        psT[:], vt[:, j4 * 128:(j4 + 1) * 128], ident[:])
                    for hs in range(HPC):
                        nc.vector.tensor_copy(
                            VT2[:, t_i * W2 + hs * 65:t_i * W2 + hs * 65 + 64],
                            psT[:, hs * 64:(hs + 1) * 64])
                    if j4 == 3:
                        del v_tiles[(bb, tcl)]
                return emit

            def make_m_unit(bb, tcl, fb, quarter, alt=False):
                """2 of the 8 accumulating QKV matmuls; evac on last quarter.
                alt: alternate psum banks (prefix only, when pp is idle).
                fb==2 (V) runs 'direct': stationary = the xT tile, moving =
                W_v, so each quarter yields a [token,vcol] tile that goes
                straight into VT2 — no PE transpose needed. All 4 quarters
                accumulate in one pm bank as a single start group (the
                start's zero region covers the whole bank)."""
                def emit():
                    UNIT_LOG.append((f"m{bb}.{tcl}.{fb}.{quarter}", nc.next_id()))
                    tci = bb * n_tcb + tcl
                    xt = xt_tiles[(bb, tcl)]
                    if quarter == 0:
                        pool = ppp if (alt and fb % 2 == 1) else pmp
                        tag = "pp" if (alt and fb % 2 == 1) else "pm"
                        pm = pool.tile([128, 512], F32, tag=tag, name="pm")
                        qkv_psum[(bb, tcl, fb)] = pm
                    elif quarter == 3:
                        pm = qkv_psum.pop((bb, tcl, fb))
                    else:
                        pm = qkv_psum[(bb, tcl, fb)]
                    for d in range(2 * quarter, 2 * quarter + 2):
                        nc.tensor.matmul(
                            pm[:], w_sb[d][:, fb * 128:(fb + 1) * 128],
                            xt[:, d * 512:(d + 1) * 512],
                            start=(d == 0), stop=(d == n_dc - 1))
                    if quarter == 3:
                        if fb == 2:
                            vt = vbp.tile([128, 512], BF16, tag="vb",
                                          name="vt")
                            v_tiles[(bb, tcl)] = vt
                            nc.vector.tensor_copy(vt[:], pm[:])
                        else:
                            dst = (Q_sb if fb == 0 else K_sb)
                            nc.vector.tensor_copy(
                                dst[:, tci * 512:(tci + 1) * 512], pm[:])
                return emit

            def make_av_unit(it_idx, b, hs, qc, qt, act_mul=False,
                             av_pool=None):
                """attn@V chain for one qtile + per-partition normalize."""
                def emit():
                    UNIT_LOG.append((f"av{b}.{qc}.{hs}.{qt}", nc.next_id()))
                    pool, tag = av_pool if av_pool else (po2p, "po2")
                    po2 = pool.tile([128, DH + 1], F32, tag=tag, name="po2")
                    t0 = b * n_kt
                    for kt in range(n_kt):
                        at = at_tiles[(it_idx, kt)]
                        nc.tensor.matmul(
                            po2[:],
                            at[:, qt * 128:(qt + 1) * 128],
                            VT2[:, (t0 + kt) * W2 + hs * 65:
                                (t0 + kt) * W2 + (hs + 1) * 65],
                            start=(kt == 0), stop=(kt == n_kt - 1))
                    if (b, qc) not in a2_tiles:
                        a2_tiles[(b, qc)] = a2p.tile(
                            [128, n_qt * 128], BF16, tag="a2", name="a2")
                    a2 = a2_tiles[(b, qc)]
                    rec = smallp.tile([128, 1], F32, tag="rc", name="rec")
                    nc.vector.reciprocal(rec[:], po2[:, DH:DH + 1])
                    dst = a2[:, qt * 128 + hs * 64:qt * 128 + hs * 64 + 64]
                    if act_mul:
                        # tail only: ACT is idle once the exps are done
                        nc.scalar.mul(dst, po2[:, 0:DH], rec[:])
                    else:
                        nc.vector.tensor_scalar_mul(dst, po2[:, 0:DH], rec[:])
                return emit

            tr_emitted = {}  # (b, qc) -> trs emitted so far

            def make_tr_unit(b, qc, qt, act_copy=False):
                """Transpose one normalized [q,c] tile into A_sb [c, tok]."""
                def emit():
                    UNIT_LOG.append((f"tr{b}.{qc}.{qt}", nc.next_id()))
                    tr_emitted[(b, qc)] = tr_emitted.get((b, qc), 0) + 1
                    a2 = a2_tiles[(b, qc)]
                    psT = po2p.tile([128, 128], BF16, tag="po2", name="psT2")
                    nc.tensor.transpose(
                        psT[:], a2[:, qt * 128:(qt + 1) * 128], ident[:])
                    dst = A_sb[:, b * SEQ + qc * QC + qt * 128:
                               b * SEQ + qc * QC + (qt + 1) * 128]
                    if act_copy:
                        nc.scalar.copy(dst, psT[:])
                    else:
                        nc.vector.tensor_copy(dst, psT[:])
                    if qt == n_qt - 1:
                        del a2_tiles[(b, qc)]
                return emit

            proj_osb = {}

            def make_proj_unit(b, et, sc, alt=False, act_evac=False,
                               rot=False):
                def emit():
                    UNIT_LOG.append((f"proj{b}.{et}.{sc}", nc.next_id()))
                    # tail (alt/rot): the ps psum slots are free once the
                    # last exp has run; cycle ppp + the two ps slots to
                    # pipeline (po2p stays exclusive to the av chains)
                    pool, tag = ppp, "pp"
                    if alt or rot:
                        pool, tag = [(ppp, "pp"), (psp, "ps"),
                                     (psp, "ps")][(et * 2 + sc) % 3]
                    pp = pool.tile([128, 512], F32, tag=tag, name="pp")
                    nc.tensor.matmul(
                        pp[:], wo_sb[:, et * 128:(et + 1) * 128],
                        A_sb[:, b * SEQ + sc * 512:b * SEQ + (sc + 1) * 512],
                        start=True, stop=True)
                    # PSUM evac: DVE, or (tail-only, once the exps are done)
                    # ACT; GPSIMD cannot read PSUM on real hardware
                    copy_fn = nc.scalar.copy if act_evac \
                        else nc.vector.tensor_copy
                    # evacuate sc-pairs into one o_sb and write both halves
                    # with a single DMA (fewer, larger part writes); in the
                    # tail (alt) the sc-halves of a pair are staged far
                    # apart, so write each half individually instead
                    if alt:
                        o_sb = osp.tile([128, 1024], BF16, tag="os",
                                        name="o_sb")
                        copy_fn(o_sb[:, 0:512], pp[:])
                        nc.sync.dma_start(
                            part_d[(b, sc // 2)][et * 128:(et + 1) * 128,
                                                 (sc % 2) * 512:
                                                 (sc % 2 + 1) * 512],
                            o_sb[:, 0:512])
                        return
                    if (b, et, sc // 2) not in proj_osb:
                        proj_osb[(b, et, sc // 2)] = osp.tile(
                            [128, 1024], BF16, tag="os", name="o_sb")
                    o_sb = proj_osb[(b, et, sc // 2)]
                    half = sc % 2
                    copy_fn(o_sb[:, half * 512:(half + 1) * 512], pp[:])
                    if half == 1:
                        nc.sync.dma_start(
                            part_d[(b, sc // 2)][et * 128:(et + 1) * 128, :],
                            o_sb[:])
                        del proj_osb[(b, et, sc // 2)]
                return emit

            def make_rs_unit(b, qc):
                def emit():
                    UNIT_LOG.append((f"rs{b}.{qc}", nc.next_id()))
                    # keep the collective/RS-replacement DMA off the Pool
                    # queue: its wait for part writes must not block proj
                    # PSUM evacuation
                    if skip_collectives:
                        nc.sync.dma_start(rs_d[(b, qc)][:],
                                          part_d[(b, qc)][0:128, :])
                    else:
                        nc.gpsimd.collective_compute(
                            "ReduceScatter", mybir.AluOpType.add,
                            replica_groups=groups,
                            ins=[part_d[(b, qc)].opt()],
                            outs=[rs_d[(b, qc)].opt()])
                    nc.sync.dma_start(
                        outT[:, b * SEQ + qc * QC:b * SEQ + (qc + 1) * QC],
                        rs_d[(b, qc)][:])
                return emit

            def proj_ready(unit):
                kind, _, b, et, sc = unit
                if kind != "proj":
                    return True
                return tr_emitted.get((b, sc // 2), 0) >= 4 * (sc % 2 + 1)

            def emit_proj(unit, alt=False, act_evac=False):
                kind, _, b, et, sc = unit
                if kind == "proj":
                    if alt and (b, et, sc // 2) in proj_osb:
                        # partner half already went through the pairing path;
                        # complete the pair so its part DMA fires
                        alt, act_evac = False, False
                    make_proj_unit(b, et, sc, alt=alt, act_evac=act_evac)()
                else:
                    make_rs_unit(b, et)()

            INF = float("inf")

            def x_front(bb, skip_x=()):
                """Lead x DMA units; staged a half-iteration before the
                matmul units so the chunk DMAs land first."""
                return [(("qkv", bb), 0, make_x_unit(bb, tcl), INF)
                        for tcl in (0, 1) if (bb, tcl) not in skip_x]

            def qkv_qk(bb):
                """Q+K m-units (scores prerequisites) with the trailing x
                DMAs; drained under the ('qkv', bb) barrier."""
                units = []
                for tcl in range(n_tcb):
                    for fb in (0, 1):
                        for q4 in range(4):
                            units.append((("qkv", bb), 427,
                                          make_m_unit(bb, tcl, fb, q4), INF))
                    if tcl + 2 < n_tcb:
                        units.append((("qkv", bb), 0,
                                      make_x_unit(bb, tcl + 2), INF))
                return units

            def qkv_vt(bb):
                """V m-units + V transposes; only needed (emission-wise)
                before the first av unit of batch bb goes out — the hot
                queue gates on vt_emitted, with a loose deadline backstop."""
                units = []
                for tcl in range(n_tcb):
                    for q4 in range(4):
                        units.append((("qkvv", bb), 427,
                                      make_m_unit(bb, tcl, 2, q4), INF))
                    for j4 in range(4):
                        units.append((("qkvv", bb), 110,
                                      make_t_unit(bb, tcl, j4), INF))
                return units

            def stage_post_iter(it_idx, b, hs, qc, hot, cold, proj_q):
                """Stage follow-up work for a finished (b, hs, qc) iter."""
                if hs == 0:
                    hot.extend((("av", b), 433,
                                make_av_unit(it_idx, b, hs, qc, qt))
                               for qt in range(n_qt))
                else:
                    # interleave av(h1,qt) with tr(qt): tr depends on it
                    for qt in range(n_qt):
                        hot.append((("av", b), 433,
                                    make_av_unit(it_idx, b, hs, qc, qt)))
                        hot.append((("tr", b), 55, make_tr_unit(b, qc, qt)))
                    # a proj of sc-half scl reads A_sb written by trs
                    # qt 4*scl..4*scl+3; gate emission on those trs being out
                    proj_q.extend(("proj", 213, b, et, qc * 2 + sc)
                                  for et in range(n_et) for sc in range(2))
                    proj_q.append(("rs", 0, b, qc, None))

            # ---- prefix: only Q(qc0) + K for batch 0 run up front; V, the
            # V-transposes and Q(qc1) are deferred into the attention loop so
            # the first exp can start ~10 us earlier ----
            PHASE_MARKS.append(("qkv", nc.next_id()))
            from collections import deque
            hot = deque()     # av/tr units: latency-critical, dep-ordered
            cold = deque()    # qkv units
            proj_q = deque()  # proj/rs chain (independent of qkv)

            for unit in x_front(0, skip_x={(0, 0)}):
                unit[2]()
            make_x_unit(0, 2)()
            make_x_unit(0, 3)()
            # wo is first read by the projs (~iter 1); keep its transfer
            # behind the x chunks the first scores wait on
            nc.sync.dma_start(wo_sb[:], wo[:])
            # only Q(qc0) + K(tcl0) ahead of the kt loop; K(tcl1..3) are
            # force-drained inside iter 0 just ahead of the kts that read
            # them, so the first exp starts ~5 us earlier still
            for tcl in range(2):
                for q4 in range(4):
                    make_m_unit(0, tcl, 0, q4, alt=True)()
            for q4 in range(4):
                make_m_unit(0, 0, 1, q4, alt=True)()
            k_pre = deque(make_m_unit(0, tcl, 1, q4, alt=True)
                          for tcl in range(1, n_tcb) for q4 in range(4))
            # deferred b0 work: V+transposes (needed before b0's avs are
            # emitted) and Q(qc1) (needed before the b0/qc1 scores)
            b0_rest = qkv_vt(0)  # [V0,t0, V1,t1, V2,t2, V3,t3] x4 units each
            for pos, tcl in ((24, 3), (16, 2)):  # Q(qc1) between V/t groups
                b0_rest[pos:pos] = [(("qkvq1", 0), 427,
                                     make_m_unit(0, tcl, 0, q4), INF)
                                    for q4 in range(4)]
            cold.extend(b0_rest)

            # ---- attention with injected background work ----
            PHASE_MARKS.append(("attn", nc.next_id()))

            def hot_ready(unit):
                # an av unit of batch b must not be emitted before all of
                # b's V-transposes have written their VT2 columns
                kind = unit[0]
                if kind[0] == "av":
                    return vt_emitted.get(kind[1], 0) >= n_tcb * 4
                return True

            iters = [(b, qc, hs) for b in range(B)
                     for qc in range(n_qc) for hs in range(HPC)]
            prev = None
            credit = 0.0
            for it_idx, (b, qc, hs) in enumerate(iters):
                if prev is not None:
                    stage_post_iter(it_idx - 1, prev[0], prev[2], prev[1],
                                    hot, cold, proj_q)
                prev = (b, qc, hs)
                if hs == 0 and qc == 0:
                    # all Q/K(b) must be emitted before scores(b) reference
                    # it; pop from the head (FIFO order preserved) until no
                    # ('qkv', b) unit remains queued
                    while any(u[0] == ("qkv", b) for u in cold):
                        cold.popleft()[2]()
                    if b + 1 < B:
                        cold.extend(x_front(b + 1))
                if b == 0 and qc == 1 and hs == 0:
                    # deferred Q(qc1) units must be out before qc1's scores
                    while any(u[0] == ("qkvq1", 0) for u in cold):
                        cold.popleft()[2]()

                if it_idx > 0:
                    # fresh av/tr staging plus carried credit would emit a
                    # multi-us PE burst before kt1's scores; clamp it
                    credit = min(credit, 300.0)
                q0 = b * SEQ + qc * QC
                hrow = hs * 64
                for kt in range(n_kt):
                    # b0/iter0: K(tcl) must be emitted before kt 4*tcl reads
                    # it; two quarters per kt keeps K one tile ahead
                    if it_idx == 0 and k_pre and kt < 12:
                        k_pre.popleft()()
                    UNIT_LOG.append((f"sc{b}.{qc}.{hs}.{kt}", nc.next_id()))
                    ps = psp.tile([128, QC], F32, tag="ps", name="ps")
                    k_stat = K_sb[hrow:hrow + 64,
                                  b * SEQ + kt * 128:b * SEQ + (kt + 1) * 128]
                    for j in range(QC // MS):
                        nc.tensor.matmul(
                            ps[:, j * MS:(j + 1) * MS], k_stat,
                            Q_sb[hrow:hrow + 64,
                                 q0 + j * MS:q0 + (j + 1) * MS],
                            start=True, stop=True)
                    at = atp.tile([128, QC], BF16, tag="at", name="at")
                    nc.scalar.activation(at[:], ps[:], EXP, scale=0.125)
                    at_tiles[(it_idx, kt)] = at
                    if hs == 0 and qc == 0 and kt == 2 and b + 1 < B:
                        # matmul units a half-iter behind their x DMAs
                        cold.extend(qkv_qk(b + 1))
                        cold.extend(qkv_vt(b + 1))
                    # drain background work into the ACT slack of this kt.
                    # credit = ACT pace minus PE work already emitted; keeps
                    # PE fed without letting it run far ahead of the exps.
                    credit = min(credit + 1038 - 427, 3000)
                    while hot and hot[0][1] <= credit + 240 \
                            and hot_ready(hot[0]):
                        unit = hot.popleft()
                        credit -= unit[1]
                        unit[2]()
                    # proj chain: up to one unit per kt (ppp's write+evac
                    # round trip is ~900 ns < the 1038 ns kt pace), two when
                    # the qkv queue is dry (late iters) to avoid a tail
                    # backlog; independent of the qkv queue
                    for _ in range(1 if cold else 2):
                        if proj_q and proj_q[0][1] <= credit + 300 \
                                and proj_ready(proj_q[0]):
                            unit = proj_q.popleft()
                            credit -= unit[1]
                            emit_proj(unit)
                    # qkv m-units hold off in the first kts of a batch so
                    # the x chunk DMAs get a head start
                    if not (hs == 0 and qc == 0 and kt < 4):
                        while cold and cold[0][1] <= credit:
                            unit = cold.popleft()
                            credit -= unit[1]
                            unit[2]()
                    else:
                        while cold and cold[0][1] == 0:
                            cold.popleft()[2]()

            # ---- tail ----
            # Critical path: the last iter's exps -> its av chains -> trs ->
            # the qc1 projs -> rs chunk. Emit the av/tr chain immediately
            # (engine FIFOs execute in emission order), slipping leftover
            # proj_q units between av groups as PE filler; each sc-half's
            # projs go out as soon as its 4 tr units are emitted, on the
            # ppp/ps psum slots so the av chain's po2 slots are never blocked.
            PHASE_MARKS.append(("tail", nc.next_id()))
            bL, qcL, hsL = prev
            itL = len(iters) - 1

            # leftover cold (V/t stragglers) first — the avs depend on their
            # emission — then leftover h0-av units: their exps are long done,
            # so they fill PE while the last h1 exps drain; the tr units
            # below read the a2 columns they write
            while cold:
                cold.popleft()[2]()
            while hot:
                hot.popleft()[2]()
            # leftover proj_q first: their PE matmuls fill the last-exp wait
            # and MUST precede the av/tr chain on the PE queue — their ACT
            # evacuations would otherwise cycle with the tr units' ACT muls
            while proj_q:
                emit_proj(proj_q.popleft(), alt=True, act_evac=True)
            # av chains prefetch their early-kt matmuls during the remaining
            # exps (po2p x2 plus the now-free pm bank = 3 in flight); each tr
            # follows two avs later in the PE FIFO so it never head-blocks;
            # normalize/copy work alternates DVE/ACT (ACT idles once the
            # exps are done). Final projs pair-write (one [128,1024] part
            # DMA per et — HWDGE charges ~625 ns per DMA regardless of
            # width), and the rs chunk goes right after its et0 write, the
            # only one the reduce-scatter's own rows wait on.
            def t_av(qt):
                return make_av_unit(
                    itL, bL, hsL, qcL, qt, act_mul=(qt % 2 == 1),
                    av_pool=[(po2p, "po2"), (po2p, "po2"),
                             (pmp, "pm")][qt % 3])

            def t_tr(qt):
                return make_tr_unit(bL, qcL, qt, act_copy=(qt % 2 == 0))

            def t_p(g, et):
                return make_proj_unit(bL, et, qcL * 2 + g, rot=True,
                                      act_evac=(et % 2 == 1))

            seq = [t_av(0), t_av(1), t_av(2), t_tr(0), t_av(3), t_tr(1),
                   t_av(4), t_tr(2), t_av(5), t_tr(3), t_av(6), t_tr(4),
                   t_p(0, 0), t_p(0, 1), t_av(7), t_tr(5), t_p(0, 2),
                   t_p(0, 3), t_tr(6), t_p(0, 4), t_p(0, 5), t_tr(7),
                   t_p(0, 6), t_p(0, 7), t_p(1, 0), t_p(1, 1), t_p(1, 2),
                   t_p(1, 3), t_p(1, 4), t_p(1, 5), t_p(1, 6), t_p(1, 7),
                   make_rs_unit(bL, qcL)]
            for u in seq:
                u()
            for unit in list(hot) + list(cold):
                unit[2]()

    nc.compile()
    return nc


def host_prep(x, W_qkv, W_o, B=4, SEQ=2048):
    """Slice/transpose full inputs into per-core input maps."""
    import ml_dtypes
    TOK = B * SEQ
    # pack as [128, d_chunk*TOK]: row r, col d*TOK+t = x[t, d*128+r]
    xT = np.ascontiguousarray(
        x.reshape(TOK, 8, 128).transpose(2, 1, 0).reshape(128, 8 * TOK)
    ).astype(ml_dtypes.bfloat16)
    in_maps = []
    for c in range(N_CORES):
        cols = []
        for part in range(3):  # q, k, v column blocks of this core's heads
            base = part * D_MODEL + c * HPC * DH
            cols.append(W_qkv[:, base:base + HPC * DH])
        wq = np.concatenate(cols, axis=1)  # [1024, 384]
        # pack as [128, d_chunk*384]: row r, col d*384+j = wq[d*128+r, j]
        wqkv_c = np.ascontiguousarray(
            wq.reshape(8, 128, 3 * HPC * DH).transpose(1, 0, 2)
            .reshape(128, 8 * 3 * HPC * DH)).astype(ml_dtypes.bfloat16)
        # this core's 128 rows of W_o (the d-dims its heads produce),
        # pre-converted to bf16 (the proj moving operand A is bf16 and the
        # backend requires matching matmul input widths)
        wo_c = np.ascontiguousarray(
            W_o[c * 128:(c + 1) * 128, :]).astype(ml_dtypes.bfloat16)
        in_maps.append({"xT": xT, "wqkv": wqkv_c, "wo": wo_c})
    return in_maps


_NC_CACHE = {}


def kernel(x, W_qkv, W_o):
    from concourse.bass_utils import run_bass_kernel_spmd
    B, SEQ, _ = x.shape
    key = (B, SEQ)
    if key not in _NC_CACHE:
        _NC_CACHE[key] = build_nc(B=B, SEQ=SEQ)
    nc = _NC_CACHE[key]
    in_maps = host_prep(np.asarray(x), np.asarray(W_qkv), np.asarray(W_o),
                        B=B, SEQ=SEQ)
    try:
        res = run_bass_kernel_spmd(nc, in_maps, list(range(N_CORES))).results
    except Exception:
        # A stale axon terminal session occasionally reports the device
        # unrecoverable on the first execution after an idle period; a
        # single retry on a fresh attempt has always succeeded.
        res = run_bass_kernel_spmd(nc, in_maps, list(range(N_CORES))).results
    outT = np.concatenate([np.asarray(res[c]["outT"]).astype(np.float32)
                           for c in range(N_CORES)], axis=0)
    return np.ascontiguousarray(outT.T).reshape(B, SEQ, D_MODEL)

